# revision 92
# baseline (speedup 1.0000x reference)
"""Trainium2 Bass kernel for BEiT-3 multiway multihead attention (v2).

Strategy
--------
8-way data parallelism over the batch: each NeuronCore computes one batch
element end to end.  Feature-major compute (transposed, [E, T]) so every
matmul contracts over the partition dimension without on-chip transposes.

v2 changes vs v1:
  * q/k/v projections run as 3-term error-compensated fp8e4 DoubleRow
    matmuls (x8@w8 + x8@wr + xr@w8), 0.75x the bf16 cost at ~bf16
    accuracy.  Weights are host-scaled by 64 into the fp8 range; the 64*64
    scores scale folds into the exp() activation scale and the 64 on v
    folds into the 1/64-scaled normalization selector.
  * mask multiply is one [128,1024] DVE op with a stride-0-broadcast mask
    operand instead of two [128,512] ops.
  * softmax denominators: reciprocal_approx_fast straight off the PSUM
    denominator row (no ACT copy + DMA round trip), per head pair, so
    normalization of each attn chunk happens during the attention phase.
  * v projection streams token-chunk-major so attention can start early.

  qT/kT = W-stationary projections (feature-major outputs)
  v     = token-major projection with an extra all-ones column per head so
          the P@V matmul also produces softmax denominators (row 64)
  scores[s, t] = (kT-slice).T @ (qT-slice) per head, fp32 in PSUM
  probs = exp(scores * scaling/4096) * em  (em = exp(mask).T, bf16)
  attn_u[hd, t] (+ denominator row) = v-slice.T @ probs
  attn = attn_u * (1/(64 d))  via a tiny K=2 f32r selector matmul
  LayerNorm folded into the output projection: Wg = Wo * gamma on host,
  mean via a rank-1 correction matmul, 1/std via a PE-broadcast row.
"""

from contextlib import ExitStack

import numpy as np
import ml_dtypes

import concourse.bass as bass
import concourse.mybir as mybir
from concourse import bacc, tile
from concourse.bass import ts
from concourse.bass_utils import run_bass_kernel_spmd

AF = mybir.ActivationFunctionType
DR = mybir.MatmulPerfMode.DoubleRow

B = 8
E = 1024
T = 1024
H = 16
HD = 64
P = 128
NCH = E // P          # feature chunks (= head pairs)
NTC = T // P          # token chunks
NKP = NCH // 2        # DoubleRow k-tile pairs
EPS = 1e-5
WS = 64.0             # host weight scale into fp8 range
BF16 = mybir.dt.bfloat16
F32 = mybir.dt.float32
F32R = mybir.dt.float32r
FP8 = mybir.dt.float8e4
NPBF16 = ml_dtypes.bfloat16
NPF8 = ml_dtypes.float8_e4m3


def _segs(lo, hi, split):
    """Token segments [lo, hi) split by modality boundary. -> [(s0, s1, m)]"""
    out = []
    if lo < min(hi, split):
        out.append((lo, min(hi, split), 0))
    if max(lo, split) < hi:
        out.append((max(lo, split), hi, 1))
    return out


def build_module(split: int, v_bias: bool, qk_bias: bool = True, o_bias: bool = True):
    assert 0 <= split <= T and split % 32 == 0, split
    nc = bacc.Bacc("TRN2", target_bir_lowering=False, debug=False)

    # x packed [P, NCH*T]: row p holds chunk-major data (chunk stride = T)
    xq8_d = nc.declare_dram_parameter("xq8", [P, NCH * T], FP8, isOutput=False)
    xqr_d = nc.declare_dram_parameter("xqr", [P, NCH * T], FP8, isOutput=False)
    xk8_d = nc.declare_dram_parameter("xk8", [P, NCH * T], FP8, isOutput=False)
    xkr_d = nc.declare_dram_parameter("xkr", [P, NCH * T], FP8, isOutput=False)
    xv8_d = nc.declare_dram_parameter("xv8", [P, NCH * T], FP8, isOutput=False)
    xvr_d = nc.declare_dram_parameter("xvr", [P, NCH * T], FP8, isOutput=False)
    # packed weights: one DMA per (side, eo) — per eo the free axis holds
    # [m0-main | m1-main | m0-residual | m1-residual], each NCH*P wide
    wq_all = nc.declare_dram_parameter("wq_all", [NCH, P, 4 * NCH * P], FP8,
                                       isOutput=False)
    wk_all = nc.declare_dram_parameter("wk_all", [NCH, P, 4 * NCH * P], FP8,
                                       isOutput=False)
    # v weights: per eoh [m0 | m1], mains and residuals separate
    wv_main = nc.declare_dram_parameter("wv_main", [2, P, 2 * NCH * 512], FP8,
                                        isOutput=False)
    wv_res = nc.declare_dram_parameter("wv_res", [2, P, 2 * NCH * 512], FP8,
                                       isOutput=False)
    # o-projection weights: per eo [m0 | m1]
    wg = nc.declare_dram_parameter("wg", [NCH, P, 2 * NCH * P], BF16,
                                   isOutput=False)
    c1p = nc.declare_dram_parameter("c1p", [NCH, 2 * P], F32R, isOutput=False)
    em = nc.declare_dram_parameter("em", [P, NCH * T], BF16, isOutput=False)
    bq = nc.declare_dram_parameter("bq", [2, E], F32, isOutput=False)
    bk = nc.declare_dram_parameter("bk", [2, E], F32, isOutput=False)
    bv = nc.declare_dram_parameter("bv", [2, E], F32R, isOutput=False)
    c1 = nc.declare_dram_parameter("c1", [2, E], F32R, isOutput=False)
    c2 = nc.declare_dram_parameter("c2", [2, E], F32, isOutput=False)
    indp_d = nc.declare_dram_parameter("indp_d", [3, P], F32R, isOutput=False)
    ind8_d = nc.declare_dram_parameter("ind8_d", [8, 4 * P], F32R, isOutput=False)
    ones8_d = nc.declare_dram_parameter("ones8_d", [H // 2, T], F32R,
                                        isOutput=False)
    outT = nc.declare_dram_parameter("outT", [E, T], F32, isOutput=True)

    used_m = sorted(set(m for _, _, m in _segs(0, T, split)))
    exp_scale = float(HD ** -0.5 / (WS * WS))

    with tile.TileContext(nc) as tc:
      with ExitStack() as ctx:
        const = ctx.enter_context(tc.tile_pool(name="const", bufs=1))
        ones_col = const.tile([P, 1], BF16)           # stats matmul lhsT
        nc.vector.memset(ones_col[:], 1.0)
        ones_row = const.tile([1, P], F32R)
        nc.sync.dma_start(ones_row[:], indp_d[2:3])
        ind8 = const.tile([8, 4 * P], F32R)           # 1/(64 d) selector
        nc.sync.dma_start(ind8[:], ind8_d[:])
        epst = const.tile([1, 1], F32)
        nc.vector.memset(epst[:], EPS)

        # biases as per-partition columns: col m*NCH+eo holds slice for chunk eo
        bq_sb = const.tile([P, 2 * NCH], F32)
        bk_sb = const.tile([P, 2 * NCH], F32)
        c2_sb = const.tile([P, 2 * NCH], F32)
        if qk_bias or o_bias:
            for m in (0, 1):
                cs = slice(m * NCH, (m + 1) * NCH)
                nc.sync.dma_start(bq_sb[:, cs], bq[m].rearrange("(c p) -> p c", p=P))
                nc.sync.dma_start(bk_sb[:, cs], bk[m].rearrange("(c p) -> p c", p=P))
                nc.sync.dma_start(c2_sb[:, cs], c2[m].rearrange("(c p) -> p c", p=P))
        # c1 slices stream per-eo during the output projection (a resident
        # [1, 2E] tile would reserve 8 KiB of SBUF column space)
        bv_row_sb = None
        if v_bias:
            bv_row_sb = const.tile([1, 2 * E], F32R)
            for m in (0, 1):
                nc.sync.dma_start(bv_row_sb[0:1, m * E:(m + 1) * E], bv[m][None, :])

        proj_ps = ctx.enter_context(tc.tile_pool(name="proj_ps", bufs=2, space="PSUM"))

        # long-lived SBUF pools
        attn_pool = ctx.enter_context(tc.tile_pool(name="attn", bufs=1))
        wg_pool = ctx.enter_context(tc.tile_pool(name="wg_sb", bufs=2))
        osb_pool = ctx.enter_context(tc.tile_pool(name="osb", bufs=3))
        sq_pool = ctx.enter_context(tc.tile_pool(name="sq_sb", bufs=1))

        # attn_t / rd tiles are allocated lazily (after the projections) to
        # keep the SBUF high-water mark down; see below.
        attn_t = [None] * NCH
        rd_half = [None, None]

        main = ExitStack()
        with main:
            qk_sb = main.enter_context(tc.tile_pool(name="qk_sb", bufs=4))
            vem_pool = main.enter_context(tc.tile_pool(name="vem", bufs=1))
            pr_pool = main.enter_context(tc.tile_pool(name="probs", bufs=3))
            rr_pool = main.enter_context(tc.tile_pool(name="rrow", bufs=1))
            x_pool = main.enter_context(tc.tile_pool(name="xpool", bufs=1))
            sc_pool = main.enter_context(
                tc.tile_pool(name="sc_ps", bufs=2, space="PSUM"))
            at_pool = main.enter_context(
                tc.tile_pool(name="at_ps", bufs=1, space="PSUM"))
            wqk_pool = main.enter_context(tc.tile_pool(name="wqk", bufs=2))

            # -------- x input tiles (fp8 main + residual) --------
            # DMAs split per k-tile pair so the first projection matmuls can
            # start as soon as the first chunk-pair lands; weight DMAs for
            # eo=0 are emitted first (emit_qk_weights below) so they are not
            # queued behind 12 MB of x traffic.
            def xtile(name, dram, defer=False):
                t = x_pool.tile([P, NCH * T], FP8, tag=name, name=name)
                if not defer:
                    for kp in range(NKP):
                        s = slice(2 * kp * T, (2 * kp + 2) * T)
                        nc.sync.dma_start(t[:, s], dram[:, s])
                return t

            def emit_qk_weights(eo, eng=None):
                # one DMA per side; returns {(name): tile}, sliced via wkt
                eng = eng or nc.gpsimd
                wt = {}
                for name, dram in (("q", wq_all), ("k", wk_all)):
                    t = wqk_pool.tile([P, 4 * NCH * P], FP8, tag=f"w{name}",
                                      name=f"w{name}{eo}")
                    eng.dma_start(t[:], dram[eo])
                    wt[name] = t
                return wt

            xq8 = xtile("xq8", xq8_d, defer=True)
            xk8 = xtile("xk8", xk8_d, defer=True)
            xqr = xtile("xqr", xqr_d, defer=True)
            xkr = xtile("xkr", xkr_d, defer=True)

            def _xdma(eng, t, dram):
                # two halves so the first k-tile pairs land early
                for h in (0, 1):
                    s = slice(h * 4 * T, (h + 1) * 4 * T)
                    eng.dma_start(t[:, s], dram[:, s])

            # startup: q-side weights+x on the Pool DGE queue, k-side on the
            # SP queue, weights issued before their x tensors.  Engine
            # queues issue independently and the shared DMA engines serve
            # requests in arrival order, so the first matmul is gated only
            # by wq + the first xq8 half.
            w_eo0 = {}
            wq0 = wqk_pool.tile([P, 4 * NCH * P], FP8, tag="wq", name="wq0")
            nc.gpsimd.dma_start(wq0[:], wq_all[0])
            w_eo0["q"] = wq0
            _xdma(nc.gpsimd, xq8, xq8_d)
            _xdma(nc.gpsimd, xqr, xqr_d)
            wk0 = wqk_pool.tile([P, 4 * NCH * P], FP8, tag="wk", name="wk0")
            nc.sync.dma_start(wk0[:], wk_all[0])
            w_eo0["k"] = wk0
            _xdma(nc.sync, xk8, xk8_d)
            _xdma(nc.sync, xkr, xkr_d)

            def xkt(xt, kp, s0, s1):
                # [P, 2, n] k-tile-pair AP over packed x (chunk stride T)
                return (xt[:, 2 * kp * T:(2 * kp + 2) * T]
                        .rearrange("p (c t) -> p c t", c=2)[:, :, s0:s1])

            def wkt(wtile, m, var, kp):
                # [P, 2, 128] k-tile-pair AP over a packed weight tile
                base = (2 * var + m) * NCH * P + 2 * kp * P
                return (wtile[:, base:base + 2 * P]
                        .rearrange("p (c m) -> p c m", c=2))

            qT_t, kT_t = [], []
            filler = []   # [(eo, closure)] in FIFO order

            def drain_filler(n=None):
                k = len(filler) if n is None else min(n, len(filler))
                for _ in range(k):
                    filler.pop(0)[1]()

            def drain_until(eo):
                # force-emit everything this pair depends on
                while filler and filler[0][0] <= eo:
                    filler.pop(0)[1]()

            def push_qk_proj(eo, wt):
                """Queue the eo projection as small closures; the attention
                loop drains them so they fill PE gaps instead of clumping at
                a pair boundary."""
                for name, x8, xr, b_sb, out_list in (
                    ("q", xq8, xqr, bq_sb, qT_t),
                    ("k", xk8, xkr, bk_sb, kT_t),
                ):
                    qtile = qk_sb.tile([P, T], BF16, tag=f"{name}T",
                                       name=f"{name}T{eo}")
                    out_list.append(qtile)
                    for half in (0, 1):
                        lo = half * 512
                        box = {}

                        def mms(name=name, half=half, lo=lo, box=box,
                                x8=x8, xr=xr):
                            # one complete start->stop accumulation group per
                            # closure: interleaved single-MM groups (db, mask)
                            # must never split an open group in this pool
                            ps = proj_ps.tile([P, 512], F32, tag="pp",
                                              name="pp")
                            box["ps"] = ps
                            # x8 terms first: the residual tensors land later
                            # and must not head-block the in-order PE queue
                            terms = ([(0, x8, kp) for kp in range(NKP)]
                                     + [(1, x8, kp) for kp in range(NKP)]
                                     + [(0, xr, kp) for kp in range(NKP)])
                            for s0, s1, m in _segs(lo, lo + 512, split):
                                for ti, (var, xop, kp) in enumerate(terms):
                                    nc.tensor.matmul(
                                        ps[:, s0 - lo:s1 - lo],
                                        wkt(wt[name], m, var, kp),
                                        xkt(xop, kp, s0, s1),
                                        start=(ti == 0),
                                        stop=(ti == len(terms) - 1),
                                        perf_mode=DR,
                                    )

                        def evac(name=name, half=half, lo=lo, box=box,
                                 qtile=qtile, b_sb=b_sb, eo=eo):
                            ps = box.pop("ps")
                            if qk_bias:
                                for s0, s1, m in _segs(lo, lo + 512, split):
                                    nc.vector.tensor_scalar_add(
                                        qtile[:, s0:s1],
                                        ps[:, s0 - lo:s1 - lo],
                                        b_sb[:, m * NCH + eo:
                                             m * NCH + eo + 1],
                                    )
                            else:
                                nc.vector.tensor_copy(qtile[:, lo:lo + 512],
                                                      ps[:])

                        filler.append((eo, mms))
                        filler.append((eo, evac))

            push_qk_proj(0, w_eo0)
            push_qk_proj(1, emit_qk_weights(1))
            drain_filler()
            # projections 2-4 go before the v-projection: their matmuls are
            # gated only on the q/k x tensors, so they keep PE fed while the
            # v inputs are still in flight (the v matmuls would otherwise
            # head-block the in-order PE queue)
            push_qk_proj(2, emit_qk_weights(2))
            drain_filler()

            # ------------- v projection (token-major, +ones col) ------------
            v_t = []
            for tc_ in range(NTC):
                vt = vem_pool.tile([P, H * 66], BF16, tag=f"v{tc_}", name=f"v{tc_}")
                nc.vector.memset(
                    vt[:].rearrange("p (g w) -> p g w", w=66)[:, :, 64:65], 1.0
                )
                v_t.append(vt)
            xvwv = ExitStack()
            with xvwv:
                xv_pool = xvwv.enter_context(tc.tile_pool(name="xv_p", bufs=1))
                wv_pool = xvwv.enter_context(tc.tile_pool(name="wv_p", bufs=1))
                xv8 = xv_pool.tile([P, NCH * T], FP8, tag="xv8", name="xv8")
                _xdma(nc.gpsimd, xv8, xv8_d)
                xvr = xv_pool.tile([P, NCH * T], FP8, tag="xvr", name="xvr")
                _xdma(nc.gpsimd, xvr, xvr_d)

                def vwkt(wt, kp):
                    # [P, 2, 512] k-tile pair AP over wv tile (chunk stride 512)
                    return (wt[:, 2 * kp * 512:(2 * kp + 2) * 512]
                            .rearrange("p (c n) -> p c n", c=2))

                def vxkt(xt, kp, s0, s1):
                    return (xt[:, 2 * kp * T:(2 * kp + 2) * T]
                            .rearrange("p (c t) -> p c t", c=2)[:, :, s0:s1])

                for eoh in (0, 1):
                    t8 = wv_pool.tile([P, 2 * NCH * 512], FP8,
                                      tag="wv8", name=f"wv8{eoh}")
                    nc.sync.dma_start(t8[:], wv_main[eoh])
                    tr = wv_pool.tile([P, 2 * NCH * 512], FP8,
                                      tag="wvr", name=f"wvr{eoh}")
                    nc.sync.dma_start(tr[:], wv_res[eoh])
                    for tc_ in range(NTC):
                        lo = tc_ * P
                        ps = proj_ps.tile([P, 512], F32, tag="pp", name="pp")
                        segs = _segs(lo, lo + P, split)
                        # a modality-split chunk cannot use a column
                        # tile_position with DoubleRow (ISA-illegal), so each
                        # modality computes the FULL chunk with its own
                        # weights into its own bank and the evacuation picks
                        # the right rows per modality
                        ps2 = {}
                        for _, _, m in segs:
                            pst = ps if m == segs[0][2] else proj_ps.tile(
                                [P, 512], F32, tag="pp", name="pp")
                            ps2[m] = pst
                            mb = m * NCH * 512
                            for kp in range(NKP):
                                terms = (
                                    (t8, vxkt(xv8, kp, lo, lo + P)),
                                    (tr, vxkt(xv8, kp, lo, lo + P)),
                                    (t8, vxkt(xvr, kp, lo, lo + P)),
                                )
                                for ti, (wt, xap) in enumerate(terms):
                                    nc.tensor.matmul(
                                        pst[:],
                                        xap,
                                        vwkt(wt[:, mb:mb + NCH * 512], kp),
                                        start=(kp == 0 and ti == 0),
                                        stop=(kp == NKP - 1 and ti == 2)
                                        and not v_bias,
                                        perf_mode=DR,
                                    )
                            if v_bias:
                                nc.tensor.matmul(
                                    pst[:],
                                    ones_row[0:1, 0:P],
                                    bv_row_sb[
                                        0:1,
                                        m * E + eoh * 512:m * E + (eoh + 1) * 512,
                                    ],
                                    start=False,
                                    stop=True,
                                )
                        for s0, s1, m in segs:
                            m0, m1 = s0 - lo, s1 - lo
                            dst = (v_t[tc_][:]
                                   .rearrange("p (g w) -> p g w", w=66)
                                   [m0:m1, 8 * eoh:8 * eoh + 8, 0:64])
                            src_ = (ps2[m][:]
                                    .rearrange("p (g w) -> p g w", w=64)
                                    [m0:m1])
                            nc.vector.tensor_copy(dst, src_)

            # ------------- em mask factor ----------
            em_tile = vem_pool.tile([P, NCH * T], BF16, tag="em", name="em")
            _xdma(nc.gpsimd, em_tile, em)
            em_t = [em_tile[:, c * T:(c + 1) * T] for c in range(NCH)]

            for c in range(NCH):
                attn_t[c] = attn_pool.tile([P, T], BF16, tag=f"attn{c}",
                                           name=f"attn{c}")
            d_half = [None, None]
            for i in (0, 1):
                d_half[i] = attn_pool.tile([H // 2, T], F32, tag=f"d{i}",
                                           name=f"d{i}")
                # rows for not-yet-finished pairs must not be NaN: the db
                # selector multiplies them by zero, and 0*NaN = NaN
                nc.vector.memset(d_half[i][:], 1.0)
                rd_half[i] = attn_pool.tile([H // 2, T], F32R, tag=f"rd{i}",
                                            name=f"rd{i}")

            # deferred normalization closures, emitted one (pair, half) late
            # so the db matmul never head-blocks the in-order PE queue
            pending_norm = []

            def emit_pending():
                while pending_norm:
                    pending_norm.pop(0)()

            sq_t = [None] * NCH

            def defer_norm(pair, half):
                g, j = pair // 4, pair % 4
                lo = half * 512

                def go():
                    db = proj_ps.tile([P, 512], F32, tag="pp", name="pp")
                    nc.tensor.matmul(
                        db[:],
                        ind8[:, j * P:(j + 1) * P],
                        rd_half[g][:, lo:lo + 512],
                    )
                    nc.vector.tensor_mul(
                        attn_t[pair][:, lo:lo + 512],
                        attn_t[pair][:, lo:lo + 512],
                        db[:],
                    )
                    if half == 1:
                        # squares for the LN stats, while attention still runs
                        sq_t[pair] = sq_pool.tile([P, T], BF16,
                                                  tag=f"sqt{pair}",
                                                  name=f"sqt{pair}")
                        nc.vector.tensor_mul(
                            sq_t[pair][:], attn_t[pair][:], attn_t[pair][:]
                        )
                pending_norm.append(go)

            # projection 3 is queued (not drained): it fills PE gaps in
            # pairs 0-1; each pair then queues pair+4's projection, so
            # filler work is spread across the whole attention phase
            push_qk_proj(3, emit_qk_weights(3))
            for pair in range(NCH):
                drain_until(pair)
                if pair >= 1 and pair + 3 < NCH:
                    push_qk_proj(pair + 3, emit_qk_weights(pair + 3))

                # -- attention for this head pair --
                hA, hB = 2 * pair, 2 * pair + 1
                for half in (0, 1):
                    lo = half * 512
                    aA = at_pool.tile([65, 512], F32, tag="attnA", name="attnA")
                    aB = at_pool.tile([65, 512], F32, tag="attnB", name="attnB")
                    for c in range(NTC):
                        sc = sc_pool.tile([P, 1024], F32, tag="sc", name="sc")
                        nc.tensor.matmul(
                            sc[:, 0:512],
                            kT_t[pair][0:HD, ts(c, P)],
                            qT_t[pair][0:HD, lo:lo + 512],
                        )
                        nc.tensor.matmul(
                            sc[:, 512:1024],
                            kT_t[pair][HD:P, ts(c, P)],
                            qT_t[pair][HD:P, lo:lo + 512],
                        )
                        pr = pr_pool.tile([P, 1024], BF16, tag="pr", name="pr")
                        nc.scalar.activation(pr[:], sc[:], AF.Exp,
                                             scale=exp_scale)
                        em_rep = (em_t[c][:, lo:lo + 512]
                                  .unsqueeze(1).broadcast_to([P, 2, 512]))
                        nc.vector.tensor_mul(
                            pr[:].rearrange("p (c n) -> p c n", c=2),
                            pr[:].rearrange("p (c n) -> p c n", c=2),
                            em_rep,
                        )
                        nc.tensor.matmul(
                            aA[:],
                            v_t[c][:, 66 * hA:66 * hA + 65],
                            pr[:, 0:512],
                            start=(c == 0),
                            stop=(c == NTC - 1),
                        )
                        nc.tensor.matmul(
                            aB[:],
                            v_t[c][:, 66 * hB:66 * hB + 65],
                            pr[:, 512:1024],
                            start=(c == 0),
                            stop=(c == NTC - 1),
                        )
                        # pace the queued projection work across the whole
                        # attention phase (~6 closures per pair)
                        if c % 2 == 0 or len(filler) > 24:
                            drain_filler(1)
                    # emit previous chunk's normalization now: its inputs are
                    # long ready, so it slots into the PE queue without
                    # blocking, ahead of this chunk's dependent ops
                    emit_pending()
                    # denominator rows: extract from PSUM row 64 on the
                    # Scalar engine (engine writes must start 32-aligned, so
                    # arbitrary rd rows are reached via DMA), then recip the
                    # whole 8-row block from the raw values (idempotent) and
                    # round to f32r for the selector matmul
                    g, j = pair // 4, pair % 4
                    dsA = rr_pool.tile([65, 512], F32, tag="dsA", name="dsA")
                    nc.scalar.copy(dsA[64:65, :], aA[64:65, :])
                    nc.gpsimd.dma_start(
                        d_half[g][2 * j:2 * j + 1, lo:lo + 512],
                        dsA[64:65, :],
                    )
                    dsB = rr_pool.tile([65, 512], F32, tag="dsB", name="dsB")
                    nc.scalar.copy(dsB[64:65, :], aB[64:65, :])
                    nc.gpsimd.dma_start(
                        d_half[g][2 * j + 1:2 * j + 2, lo:lo + 512],
                        dsB[64:65, :],
                    )
                    rdt = rr_pool.tile([H // 2, 512], F32, tag="rdt",
                                       name="rdt")
                    nc.vector.reciprocal_approx_fast(
                        out=rdt[:], in_=d_half[g][:, lo:lo + 512],
                    )
                    nc.vector.tensor_copy(rd_half[g][:, lo:lo + 512], rdt[:])
                    # evacuate unnormalized attn on the Scalar engine: it
                    # has slack, and the DVE queue (em-mul, recips) would
                    # delay the PSUM release that gates the next half's PV
                    nc.scalar.copy(attn_t[pair][0:HD, lo:lo + 512],
                                   aA[0:HD, :])
                    nc.scalar.copy(attn_t[pair][HD:P, lo:lo + 512],
                                   aB[0:HD, :])
                    defer_norm(pair, half)
            emit_pending()
            drain_filler()

        # ---------------- LN statistics -------------------------
        def emit_wg(eo):
            wtile = wg_pool.tile([P, 2 * NCH * P], BF16, tag="wg", name="wg")
            nc.sync.dma_start(wtile[:], wg[eo])
            c1t = wg_pool.tile([1, 2 * P], F32R, tag="c1", name="c1")
            nc.sync.dma_start(c1t[:], c1p[eo][None, :])
            return {"w": wtile, "c1": c1t}

        wg_next = emit_wg(0)
        stats_pool = ctx.enter_context(tc.tile_pool(name="stats", bufs=1))
        mu_neg = stats_pool.tile([1, T], F32, tag="mu_neg", name="mu_neg")
        msq = stats_pool.tile([1, T], F32, tag="msq", name="msq")
        var = stats_pool.tile([1, T], F32, tag="var", name="var")
        rstd = stats_pool.tile([1, T], F32, tag="rstd", name="rstd")
        rstdr = stats_pool.tile([1, T], F32R, tag="rstdr", name="rstdr")
        mu_negr = stats_pool.tile([1, T], F32R, tag="mu_negr", name="mu_negr")
        rstd_bc = stats_pool.tile([P, T], F32, tag="rstd_bc", name="rstd_bc")

        with tc.tile_pool(name="db_ps", bufs=4, space="PSUM") as db_pool, \
             tc.tile_pool(name="st_ps", bufs=1, space="PSUM") as st_pool:
            # mu accumulates at partition 0, sq at partition 32 of the same
            # bank (distinct col groups) — two banks total for the stats
            stt = [st_pool.tile([33, 512], F32, tag=f"st{h}", name=f"st{h}")
                   for h in (0, 1)]
            mu_ps = [stt[h][0:1, :] for h in (0, 1)]
            sq_ps = [stt[h][32:33, :] for h in (0, 1)]
            for c in range(NCH):
                for half in (0, 1):
                    lo = half * 512
                    nc.tensor.matmul(
                        mu_ps[half], ones_col[:], attn_t[c][:, lo:lo + 512],
                        start=(c == 0), stop=(c == NCH - 1),
                    )
                    nc.tensor.matmul(
                        sq_ps[half], ones_col[:], sq_t[c][:, lo:lo + 512],
                        start=(c == 0), stop=(c == NCH - 1),
                    )
            for half in (0, 1):
                lo = half * 512
                nc.scalar.mul(mu_neg[0:1, lo:lo + 512], mu_ps[half], -1.0 / E)
                nc.scalar.mul(msq[0:1, lo:lo + 512], sq_ps[half], 1.0 / E)
            nc.vector.tensor_mul(var[:], mu_neg[:], mu_neg[:])
            nc.vector.tensor_tensor(
                var[:], msq[:], var[:], mybir.AluOpType.subtract
            )
            nc.scalar.activation(rstd[:], var[:], AF.Sqrt, bias=epst[:])
            nc.vector.reciprocal_approx_fast(out=rstd[:], in_=rstd[:])
            nc.vector.tensor_copy(rstdr[:], rstd[:])
            nc.vector.tensor_copy(mu_negr[:], mu_neg[:])

            # ---------------- output projection -----------------------------
            # the rstd broadcast matmuls are emitted after eo 0's groups so
            # the o-proj matmuls (which don't need rstd) hide the serial LN
            # scalar chain instead of head-blocking behind the broadcast
            def emit_rstd_bc():
                for half in (0, 1):
                    lo = half * 512
                    rb = db_pool.tile([P, 512], F32, tag="db", name="db")
                    nc.tensor.matmul(
                        rb[:],
                        ones_row[:],
                        rstdr[0:1, lo:lo + 512],
                    )
                    nc.vector.tensor_copy(rstd_bc[:, lo:lo + 512], rb[:])

            deferred_muls = []
            for eo in range(NCH):
                wt = wg_next
                if eo + 1 < NCH:
                    wg_next = emit_wg(eo + 1)

                for half in (0, 1):
                    lo = half * 512
                    ps = db_pool.tile([P, 512], F32, tag="db", name="db")
                    for s0, s1, m in _segs(lo, lo + 512, split):
                        for c in range(NCH):
                            nc.tensor.matmul(
                                ps[:, s0 - lo:s1 - lo],
                                wt["w"][:, m * NCH * P + c * P:
                                        m * NCH * P + (c + 1) * P],
                                attn_t[c][:, s0:s1],
                                start=(c == 0),
                                stop=False,
                            )
                        nc.tensor.matmul(
                            ps[:, s0 - lo:s1 - lo],
                            wt["c1"][0:1, m * P:(m + 1) * P],
                            mu_negr[0:1, s0:s1],
                            start=False,
                            stop=True,
                        )
                    osb = osb_pool.tile([P, 512], F32, tag="osb",
                                        name="osb")

                    def _mul(ps=ps, lo=lo, osb=osb, eo=eo):
                        nc.vector.tensor_mul(
                            osb[:], ps[:], rstd_bc[:, lo:lo + 512]
                        )
                        if o_bias:
                            for s0, s1, m in _segs(lo, lo + 512, split):
                                nc.scalar.activation(
                                    osb[:, s0 - lo:s1 - lo],
                                    osb[:, s0 - lo:s1 - lo], AF.Identity,
                                    bias=c2_sb[:, m * NCH + eo:
                                               m * NCH + eo + 1],
                                )
                        nc.sync.dma_start(outT[ts(eo, P), lo:lo + 512],
                                          osb[:])
                    if eo == 0:
                        deferred_muls.append(_mul)
                        if half == 1:
                            emit_rstd_bc()
                            while deferred_muls:
                                deferred_muls.pop(0)()
                    else:
                        _mul()


    nc.compile()
    return nc


def _pack_pmajor(arr2d, np_dt):
    # [NCH*P, T] -> [P, NCH*T]: row p holds chunk-major concatenation
    return np.ascontiguousarray(
        arr2d.reshape(NCH, P, T).transpose(1, 0, 2).reshape(P, NCH * T)
    ).astype(np_dt)


def _fp8_pair(arr):
    a8 = arr.astype(NPF8)
    ar = (arr - a8.astype(np.float32)).astype(NPF8)
    return a8, ar


def _host_prep(inputs):
    scaling = HD ** -0.5
    f32 = np.float32

    def a(name):
        return np.asarray(inputs[name], f32)

    def prep_blocks_fp8(Wt, Wi):
        # [2, eo, p, c*128+j] with arr[c*128+p, eo*128+j], fp8 main+residual
        o8 = np.empty((2, NCH, P, NCH * P), NPF8)
        orr = np.empty((2, NCH, P, NCH * P), NPF8)
        for m, W in enumerate((Wt, Wi)):
            arr = (W * WS).T.astype(f32)  # [e_in, e_out], scaled
            a8, ar = _fp8_pair(arr)
            for dst, src in ((o8, a8), (orr, ar)):
                dst[m] = (
                    src.reshape(NCH, P, NCH, P)
                    .transpose(2, 1, 0, 3)
                    .reshape(NCH, P, NCH * P)
                )
        return np.ascontiguousarray(o8), np.ascontiguousarray(orr)

    def prep_blocks_bf16(Wt, Wi):
        out = np.empty((2, NCH, P, NCH * P), NPBF16)
        for m, W in enumerate((Wt, Wi)):
            arr = (W.T).astype(NPBF16)
            out[m] = (
                arr.reshape(NCH, P, NCH, P)
                .transpose(2, 1, 0, 3)
                .reshape(NCH, P, NCH * P)
            )
        return np.ascontiguousarray(out)

    Wo_t, Wo_i = a("Wo_t"), a("Wo_i")
    g_t, g_i = a("ln_g_t"), a("ln_g_i")
    b_t, b_i = a("ln_b_t"), a("ln_b_i")
    Wg_t = Wo_t * g_t[None, :]
    Wg_i = Wo_i * g_i[None, :]

    def pack4(o8, orr):
        # [2(m), NCH, P, NCH*P] x2 -> [NCH, P, 4*NCH*P]:
        # per eo [m0-main | m1-main | m0-res | m1-res]
        return np.ascontiguousarray(
            np.concatenate([o8[0], o8[1], orr[0], orr[1]], axis=-1)
        )

    wq_np = pack4(*prep_blocks_fp8(a("Wq_t"), a("Wq_i")))
    wk_np = pack4(*prep_blocks_fp8(a("Wk_t"), a("Wk_i")))
    wg2 = prep_blocks_bf16(Wg_t, Wg_i)
    wg_np = np.ascontiguousarray(np.concatenate([wg2[0], wg2[1]], axis=-1))

    wv8_np = np.empty((2, 2, P, NCH * 512), NPF8)
    wvr_np = np.empty((2, 2, P, NCH * 512), NPF8)
    for m, W in enumerate((a("Wv_t"), a("Wv_i"))):
        arr = (W * WS).T.astype(f32)  # [e_in, e_out], scaled
        a8, ar = _fp8_pair(arr)
        for dst, src in ((wv8_np, a8), (wvr_np, ar)):
            dst[m] = (
                src.reshape(NCH, P, 2, 512)
                .transpose(2, 1, 0, 3)
                .reshape(2, P, NCH * 512)
            )
    # [eoh, P, 2*NCH*512]: per eoh [m0 | m1]
    wv_main_np = np.ascontiguousarray(
        np.concatenate([wv8_np[0], wv8_np[1]], axis=-1)
    )
    wv_res_np = np.ascontiguousarray(
        np.concatenate([wvr_np[0], wvr_np[1]], axis=-1)
    )

    em_np = _pack_pmajor(
        np.exp(np.asarray(inputs["attention_mask"], np.float64)).T.astype(NPBF16),
        NPBF16,
    )

    bq_np = np.stack([a("bq_t"), a("bq_i")]) * f32(WS)
    bk_np = np.stack([a("bk_t"), a("bk_i")]) * f32(WS)
    bv_np = np.stack([a("bv_t"), a("bv_i")]) * f32(WS)
    c1_np = np.stack(
        [Wg_t.astype(np.float64).sum(1), Wg_i.astype(np.float64).sum(1)]
    ).astype(f32)
    # [NCH, 2*P]: per eo [m0-slice | m1-slice]
    c1p_np = np.ascontiguousarray(
        np.concatenate(
            [c1_np[0].reshape(NCH, P), c1_np[1].reshape(NCH, P)], axis=-1
        )
    )
    c2_np = np.stack(
        [
            Wo_t.astype(np.float64) @ b_t.astype(np.float64) + a("bo_t"),
            Wo_i.astype(np.float64) @ b_i.astype(np.float64) + a("bo_i"),
        ]
    ).astype(f32)

    # indp row 2: all-ones (rstd bcast); ind8: 1/(64 d) selector —
    # ind8[k, j*P+m] selects rd rows (2j, 2j+1) -> bcast rows (<64, >=64)
    ones8_np = np.ones((H // 2, T), np.float32)
    indp_np = np.zeros((3, P), np.float32)
    indp_np[2, :] = 1.0
    ind8_np = np.zeros((8, 4 * P), np.float32)
    for j in range(4):
        ind8_np[2 * j, j * P:j * P + HD] = 1.0 / WS
        ind8_np[2 * j + 1, j * P + HD:(j + 1) * P] = 1.0 / WS

    shared = dict(
        wq_all=wq_np, wk_all=wk_np, wg=wg_np,
        wv_main=wv_main_np, wv_res=wv_res_np,
        em=em_np, indp_d=indp_np, ind8_d=ind8_np, ones8_d=ones8_np,
        bq=np.ascontiguousarray(bq_np), bk=np.ascontiguousarray(bk_np),
        bv=np.ascontiguousarray(bv_np), c1=np.ascontiguousarray(c1_np),
        c1p=c1p_np, c2=np.ascontiguousarray(c2_np),
    )
    flags = (
        bool(np.any(bv_np)),
        bool(np.any(bq_np) or np.any(bk_np)),
        bool(np.any(c2_np)),
    )
    return shared, flags


_CACHE = {}


def build_cached(split, flags):
    key = (split, flags)
    if key not in _CACHE:
        _CACHE[key] = build_module(split, *flags)
    return _CACHE[key]


def kernel(**inputs):
    q = np.asarray(inputs["query"], np.float32)
    k = np.asarray(inputs["key"], np.float32)
    v = np.asarray(inputs["value"], np.float32)
    assert q.shape == (B, T, E), q.shape
    split = int(np.asarray(inputs["split_position"]))

    shared, flags = _host_prep(inputs)
    nc = build_cached(split, flags)

    in_maps = []
    for b in range(B):
        m = dict(shared)
        for nm, arr in (("xq", q), ("xk", k), ("xv", v)):
            xt = arr[b].T.astype(np.float32)
            x8 = xt.astype(NPF8)
            xr = (xt - x8.astype(np.float32)).astype(NPF8)
            m[f"{nm}8"] = _pack_pmajor(x8.astype(np.float32), NPF8)
            m[f"{nm}r"] = _pack_pmajor(xr.astype(np.float32), NPF8)
        in_maps.append(m)

    res = run_bass_kernel_spmd(nc, in_maps, list(range(B)))
    out = np.stack(
        [np.ascontiguousarray(res.results[b]["outT"].T) for b in range(B)]
    )
    return out.astype(np.float32)


# revision 95
# speedup vs baseline: 1.0518x; 1.0518x over previous
"""Trainium2 Bass kernel for BEiT-3 multiway multihead attention (v2).

Strategy
--------
8-way data parallelism over the batch: each NeuronCore computes one batch
element end to end.  Feature-major compute (transposed, [E, T]) so every
matmul contracts over the partition dimension without on-chip transposes.

v2 changes vs v1:
  * q/k/v projections run as 3-term error-compensated fp8e4 DoubleRow
    matmuls (x8@w8 + x8@wr + xr@w8), 0.75x the bf16 cost at ~bf16
    accuracy.  Weights are host-scaled by 64 into the fp8 range; the 64*64
    scores scale folds into the exp() activation scale and the 64 on v
    folds into the 1/64-scaled normalization selector.
  * mask multiply is one [128,1024] DVE op with a stride-0-broadcast mask
    operand instead of two [128,512] ops.
  * softmax denominators: reciprocal_approx_fast straight off the PSUM
    denominator row (no ACT copy + DMA round trip), per head pair, so
    normalization of each attn chunk happens during the attention phase.
  * v projection streams token-chunk-major so attention can start early.

  qT/kT = W-stationary projections (feature-major outputs)
  v     = token-major projection with an extra all-ones column per head so
          the P@V matmul also produces softmax denominators (row 64)
  scores[s, t] = (kT-slice).T @ (qT-slice) per head, fp32 in PSUM
  probs = exp(scores * scaling/4096) * em  (em = exp(mask).T, bf16)
  attn_u[hd, t] (+ denominator row) = v-slice.T @ probs
  attn = attn_u * (1/(64 d))  via a tiny K=2 f32r selector matmul
  LayerNorm folded into the output projection: Wg = Wo * gamma on host,
  mean via a rank-1 correction matmul, 1/std via a PE-broadcast row.
"""

from contextlib import ExitStack

import numpy as np
import ml_dtypes

import concourse.bass as bass
import concourse.mybir as mybir
from concourse import bacc, tile
from concourse.bass import ts
from concourse.bass_utils import run_bass_kernel_spmd

AF = mybir.ActivationFunctionType
DR = mybir.MatmulPerfMode.DoubleRow

B = 8
E = 1024
T = 1024
H = 16
HD = 64
P = 128
NCH = E // P          # feature chunks (= head pairs)
NTC = T // P          # token chunks
NKP = NCH // 2        # DoubleRow k-tile pairs
EPS = 1e-5
WS = 64.0             # host weight scale into fp8 range
BF16 = mybir.dt.bfloat16
F32 = mybir.dt.float32
F32R = mybir.dt.float32r
FP8 = mybir.dt.float8e4
NPBF16 = ml_dtypes.bfloat16
NPF8 = ml_dtypes.float8_e4m3


def _segs(lo, hi, split):
    """Token segments [lo, hi) split by modality boundary. -> [(s0, s1, m)]"""
    out = []
    if lo < min(hi, split):
        out.append((lo, min(hi, split), 0))
    if max(lo, split) < hi:
        out.append((max(lo, split), hi, 1))
    return out


def build_module(split: int, v_bias: bool, qk_bias: bool = True, o_bias: bool = True):
    assert 0 <= split <= T and split % 32 == 0, split
    nc = bacc.Bacc("TRN2", target_bir_lowering=False, debug=False)

    # x packed [P, NCH*T]: row p holds chunk-major data (chunk stride = T)
    xq8_d = nc.declare_dram_parameter("xq8", [P, NCH * T], FP8, isOutput=False)
    xqr_d = nc.declare_dram_parameter("xqr", [P, NCH * T], FP8, isOutput=False)
    xk8_d = nc.declare_dram_parameter("xk8", [P, NCH * T], FP8, isOutput=False)
    xkr_d = nc.declare_dram_parameter("xkr", [P, NCH * T], FP8, isOutput=False)
    xv8_d = nc.declare_dram_parameter("xv8", [P, NCH * T], FP8, isOutput=False)
    xvr_d = nc.declare_dram_parameter("xvr", [P, NCH * T], FP8, isOutput=False)
    # packed weights: one DMA per (side, eo) — per eo the free axis holds
    # [m0-main | m1-main | m0-residual | m1-residual], each NCH*P wide
    wq_all = nc.declare_dram_parameter("wq_all", [NCH, P, 4 * NCH * P], FP8,
                                       isOutput=False)
    wk_all = nc.declare_dram_parameter("wk_all", [NCH, P, 4 * NCH * P], FP8,
                                       isOutput=False)
    # v weights: per eoh [m0 | m1], mains and residuals separate
    wv_main = nc.declare_dram_parameter("wv_main", [2, P, 2 * NCH * 512], FP8,
                                        isOutput=False)
    wv_res = nc.declare_dram_parameter("wv_res", [2, P, 2 * NCH * 512], FP8,
                                       isOutput=False)
    # o-projection weights: per eo [m0 | m1]
    wg = nc.declare_dram_parameter("wg", [NCH, P, 2 * NCH * P], BF16,
                                   isOutput=False)
    c1p = nc.declare_dram_parameter("c1p", [NCH, 2 * P], F32R, isOutput=False)
    em = nc.declare_dram_parameter("em", [P, NCH * T], BF16, isOutput=False)
    bq = nc.declare_dram_parameter("bq", [2, E], F32, isOutput=False)
    bk = nc.declare_dram_parameter("bk", [2, E], F32, isOutput=False)
    bv = nc.declare_dram_parameter("bv", [2, E], F32R, isOutput=False)
    c1 = nc.declare_dram_parameter("c1", [2, E], F32R, isOutput=False)
    c2 = nc.declare_dram_parameter("c2", [2, E], F32, isOutput=False)
    indp_d = nc.declare_dram_parameter("indp_d", [3, P], F32R, isOutput=False)
    ind8_d = nc.declare_dram_parameter("ind8_d", [8, 4 * P], F32R, isOutput=False)
    ones8_d = nc.declare_dram_parameter("ones8_d", [H // 2, T], F32R,
                                        isOutput=False)
    outT = nc.declare_dram_parameter("outT", [E, T], F32, isOutput=True)

    used_m = sorted(set(m for _, _, m in _segs(0, T, split)))
    exp_scale = float(HD ** -0.5 / (WS * WS))

    with tile.TileContext(nc) as tc:
      with ExitStack() as ctx:
        const = ctx.enter_context(tc.tile_pool(name="const", bufs=1))
        ones_col = const.tile([P, 1], BF16)           # stats matmul lhsT
        nc.vector.memset(ones_col[:], 1.0)
        ones_row = const.tile([1, P], F32R)
        nc.sync.dma_start(ones_row[:], indp_d[2:3])
        ind8 = const.tile([8, 4 * P], F32R)           # 1/(64 d) selector
        nc.sync.dma_start(ind8[:], ind8_d[:])
        epst = const.tile([1, 1], F32)
        nc.vector.memset(epst[:], EPS)

        # biases as per-partition columns: col m*NCH+eo holds slice for chunk eo
        bq_sb = const.tile([P, 2 * NCH], F32)
        bk_sb = const.tile([P, 2 * NCH], F32)
        c2_sb = const.tile([P, 2 * NCH], F32)
        if qk_bias or o_bias:
            for m in (0, 1):
                cs = slice(m * NCH, (m + 1) * NCH)
                nc.sync.dma_start(bq_sb[:, cs], bq[m].rearrange("(c p) -> p c", p=P))
                nc.sync.dma_start(bk_sb[:, cs], bk[m].rearrange("(c p) -> p c", p=P))
                nc.sync.dma_start(c2_sb[:, cs], c2[m].rearrange("(c p) -> p c", p=P))
        # c1 slices stream per-eo during the output projection (a resident
        # [1, 2E] tile would reserve 8 KiB of SBUF column space)
        bv_row_sb = None
        if v_bias:
            bv_row_sb = const.tile([1, 2 * E], F32R)
            for m in (0, 1):
                nc.sync.dma_start(bv_row_sb[0:1, m * E:(m + 1) * E], bv[m][None, :])

        proj_ps = ctx.enter_context(tc.tile_pool(name="proj_ps", bufs=2, space="PSUM"))

        # long-lived SBUF pools
        attn_pool = ctx.enter_context(tc.tile_pool(name="attn", bufs=1))
        wg_pool = ctx.enter_context(tc.tile_pool(name="wg_sb", bufs=2))
        osb_pool = ctx.enter_context(tc.tile_pool(name="osb", bufs=3))
        sq_pool = ctx.enter_context(tc.tile_pool(name="sq_sb", bufs=1))

        # attn_t / rd tiles are allocated lazily (after the projections) to
        # keep the SBUF high-water mark down; see below.
        attn_t = [None] * NCH
        rd_half = [None, None]

        main = ExitStack()
        with main:
            qk_sb = main.enter_context(tc.tile_pool(name="qk_sb", bufs=4))
            vem_pool = main.enter_context(tc.tile_pool(name="vem", bufs=1))
            pr_pool = main.enter_context(tc.tile_pool(name="probs", bufs=3))
            rr_pool = main.enter_context(tc.tile_pool(name="rrow", bufs=1))
            x_pool = main.enter_context(tc.tile_pool(name="xpool", bufs=1))
            sc_pool = main.enter_context(
                tc.tile_pool(name="sc_ps", bufs=2, space="PSUM"))
            at_pool = main.enter_context(
                tc.tile_pool(name="at_ps", bufs=1, space="PSUM"))
            wqk_pool = main.enter_context(tc.tile_pool(name="wqk", bufs=2))

            # -------- x input tiles (fp8 main + residual) --------
            # DMAs split per k-tile pair so the first projection matmuls can
            # start as soon as the first chunk-pair lands; weight DMAs for
            # eo=0 are emitted first (emit_qk_weights below) so they are not
            # queued behind 12 MB of x traffic.
            def xtile(name, dram, defer=False):
                t = x_pool.tile([P, NCH * T], FP8, tag=name, name=name)
                if not defer:
                    for kp in range(NKP):
                        s = slice(2 * kp * T, (2 * kp + 2) * T)
                        nc.sync.dma_start(t[:, s], dram[:, s])
                return t

            def emit_qk_weights(eo, eng=None):
                # one DMA per side; returns {(name): tile}, sliced via wkt
                eng = eng or nc.gpsimd
                wt = {}
                for name, dram in (("q", wq_all), ("k", wk_all)):
                    t = wqk_pool.tile([P, 4 * NCH * P], FP8, tag=f"w{name}",
                                      name=f"w{name}{eo}")
                    eng.dma_start(t[:], dram[eo])
                    wt[name] = t
                return wt

            xq8 = xtile("xq8", xq8_d, defer=True)
            xk8 = xtile("xk8", xk8_d, defer=True)
            xqr = xtile("xqr", xqr_d, defer=True)
            xkr = xtile("xkr", xkr_d, defer=True)

            def _xdma(eng, t, dram):
                # two halves so the first k-tile pairs land early
                for h in (0, 1):
                    s = slice(h * 4 * T, (h + 1) * 4 * T)
                    eng.dma_start(t[:, s], dram[:, s])

            # startup: q-side weights+x on the Pool DGE queue, k-side on the
            # SP queue, weights issued before their x tensors.  Engine
            # queues issue independently and the shared DMA engines serve
            # requests in arrival order, so the first matmul is gated only
            # by wq + the first xq8 half.
            w_eo0 = {}
            wq0 = wqk_pool.tile([P, 4 * NCH * P], FP8, tag="wq", name="wq0")
            nc.gpsimd.dma_start(wq0[:], wq_all[0])
            w_eo0["q"] = wq0
            _xdma(nc.gpsimd, xq8, xq8_d)
            _xdma(nc.gpsimd, xqr, xqr_d)
            wk0 = wqk_pool.tile([P, 4 * NCH * P], FP8, tag="wk", name="wk0")
            nc.sync.dma_start(wk0[:], wk_all[0])
            w_eo0["k"] = wk0
            _xdma(nc.sync, xk8, xk8_d)
            _xdma(nc.sync, xkr, xkr_d)

            def xkt(xt, kp, s0, s1):
                # [P, 2, n] k-tile-pair AP over packed x (chunk stride T)
                return (xt[:, 2 * kp * T:(2 * kp + 2) * T]
                        .rearrange("p (c t) -> p c t", c=2)[:, :, s0:s1])

            def wkt(wtile, m, var, kp):
                # [P, 2, 128] k-tile-pair AP over a packed weight tile
                base = (2 * var + m) * NCH * P + 2 * kp * P
                return (wtile[:, base:base + 2 * P]
                        .rearrange("p (c m) -> p c m", c=2))

            qT_t, kT_t = [], []
            filler = []   # [(eo, closure)] in FIFO order

            def drain_filler(n=None):
                k = len(filler) if n is None else min(n, len(filler))
                for _ in range(k):
                    filler.pop(0)[1]()

            def drain_until(eo):
                # force-emit everything this pair depends on
                while filler and filler[0][0] <= eo:
                    filler.pop(0)[1]()

            def push_qk_proj(eo, wt):
                """Queue the eo projection as small closures; the attention
                loop drains them so they fill PE gaps instead of clumping at
                a pair boundary."""
                for name, x8, xr, b_sb, out_list in (
                    ("q", xq8, xqr, bq_sb, qT_t),
                    ("k", xk8, xkr, bk_sb, kT_t),
                ):
                    qtile = qk_sb.tile([P, T], BF16, tag=f"{name}T",
                                       name=f"{name}T{eo}")
                    out_list.append(qtile)
                    for half in (0, 1):
                        lo = half * 512
                        box = {}

                        def mms(name=name, half=half, lo=lo, box=box,
                                x8=x8, xr=xr):
                            # one complete start->stop accumulation group per
                            # closure: interleaved single-MM groups (db, mask)
                            # must never split an open group in this pool
                            ps = proj_ps.tile([P, 512], F32, tag="pp",
                                              name="pp")
                            box["ps"] = ps
                            # x8 terms first: the residual tensors land later
                            # and must not head-block the in-order PE queue
                            terms = ([(0, x8, kp) for kp in range(NKP)]
                                     + [(1, x8, kp) for kp in range(NKP)]
                                     + [(0, xr, kp) for kp in range(NKP)])
                            for s0, s1, m in _segs(lo, lo + 512, split):
                                for ti, (var, xop, kp) in enumerate(terms):
                                    nc.tensor.matmul(
                                        ps[:, s0 - lo:s1 - lo],
                                        wkt(wt[name], m, var, kp),
                                        xkt(xop, kp, s0, s1),
                                        start=(ti == 0),
                                        stop=(ti == len(terms) - 1),
                                        perf_mode=DR,
                                    )

                        def evac(name=name, half=half, lo=lo, box=box,
                                 qtile=qtile, b_sb=b_sb, eo=eo):
                            ps = box.pop("ps")
                            if qk_bias:
                                for s0, s1, m in _segs(lo, lo + 512, split):
                                    nc.vector.tensor_scalar_add(
                                        qtile[:, s0:s1],
                                        ps[:, s0 - lo:s1 - lo],
                                        b_sb[:, m * NCH + eo:
                                             m * NCH + eo + 1],
                                    )
                            else:
                                nc.vector.tensor_copy(qtile[:, lo:lo + 512],
                                                      ps[:])

                        filler.append((eo, mms))
                        filler.append((eo, evac))

            push_qk_proj(0, w_eo0)
            push_qk_proj(1, emit_qk_weights(1))
            drain_filler()
            # projections 2-4 go before the v-projection: their matmuls are
            # gated only on the q/k x tensors, so they keep PE fed while the
            # v inputs are still in flight (the v matmuls would otherwise
            # head-block the in-order PE queue)
            push_qk_proj(2, emit_qk_weights(2))
            drain_filler()

            # ------------- v projection (token-major, +ones col) ------------
            v_t = []
            for tc_ in range(NTC):
                vt = vem_pool.tile([P, H * 66], BF16, tag=f"v{tc_}", name=f"v{tc_}")
                nc.vector.memset(
                    vt[:].rearrange("p (g w) -> p g w", w=66)[:, :, 64:65], 1.0
                )
                v_t.append(vt)
            xvwv = ExitStack()
            with xvwv:
                xv_pool = xvwv.enter_context(tc.tile_pool(name="xv_p", bufs=1))
                wv_pool = xvwv.enter_context(tc.tile_pool(name="wv_p", bufs=1))
                xv8 = xv_pool.tile([P, NCH * T], FP8, tag="xv8", name="xv8")
                _xdma(nc.gpsimd, xv8, xv8_d)
                xvr = xv_pool.tile([P, NCH * T], FP8, tag="xvr", name="xvr")
                _xdma(nc.gpsimd, xvr, xvr_d)

                def vwkt(wt, kp):
                    # [P, 2, 512] k-tile pair AP over wv tile (chunk stride 512)
                    return (wt[:, 2 * kp * 512:(2 * kp + 2) * 512]
                            .rearrange("p (c n) -> p c n", c=2))

                def vxkt(xt, kp, s0, s1):
                    return (xt[:, 2 * kp * T:(2 * kp + 2) * T]
                            .rearrange("p (c t) -> p c t", c=2)[:, :, s0:s1])

                for eoh in (0, 1):
                    t8 = wv_pool.tile([P, 2 * NCH * 512], FP8,
                                      tag="wv8", name=f"wv8{eoh}")
                    nc.sync.dma_start(t8[:], wv_main[eoh])
                    tr = wv_pool.tile([P, 2 * NCH * 512], FP8,
                                      tag="wvr", name=f"wvr{eoh}")
                    nc.sync.dma_start(tr[:], wv_res[eoh])
                    for tc_ in range(NTC):
                        lo = tc_ * P
                        ps = proj_ps.tile([P, 512], F32, tag="pp", name="pp")
                        segs = _segs(lo, lo + P, split)
                        # a modality-split chunk cannot use a column
                        # tile_position with DoubleRow (ISA-illegal), so each
                        # modality computes the FULL chunk with its own
                        # weights into its own bank and the evacuation picks
                        # the right rows per modality
                        ps2 = {}
                        for _, _, m in segs:
                            pst = ps if m == segs[0][2] else proj_ps.tile(
                                [P, 512], F32, tag="pp", name="pp")
                            ps2[m] = pst
                            mb = m * NCH * 512
                            for kp in range(NKP):
                                terms = (
                                    (t8, vxkt(xv8, kp, lo, lo + P)),
                                    (tr, vxkt(xv8, kp, lo, lo + P)),
                                    (t8, vxkt(xvr, kp, lo, lo + P)),
                                )
                                for ti, (wt, xap) in enumerate(terms):
                                    nc.tensor.matmul(
                                        pst[:],
                                        xap,
                                        vwkt(wt[:, mb:mb + NCH * 512], kp),
                                        start=(kp == 0 and ti == 0),
                                        stop=(kp == NKP - 1 and ti == 2)
                                        and not v_bias,
                                        perf_mode=DR,
                                    )
                            if v_bias:
                                nc.tensor.matmul(
                                    pst[:],
                                    ones_row[0:1, 0:P],
                                    bv_row_sb[
                                        0:1,
                                        m * E + eoh * 512:m * E + (eoh + 1) * 512,
                                    ],
                                    start=False,
                                    stop=True,
                                )
                        for s0, s1, m in segs:
                            m0, m1 = s0 - lo, s1 - lo
                            dst = (v_t[tc_][:]
                                   .rearrange("p (g w) -> p g w", w=66)
                                   [m0:m1, 8 * eoh:8 * eoh + 8, 0:64])
                            src_ = (ps2[m][:]
                                    .rearrange("p (g w) -> p g w", w=64)
                                    [m0:m1])
                            nc.vector.tensor_copy(dst, src_)

            # ------------- em mask factor ----------
            em_tile = vem_pool.tile([P, NCH * T], BF16, tag="em", name="em")
            _xdma(nc.gpsimd, em_tile, em)
            em_t = [em_tile[:, c * T:(c + 1) * T] for c in range(NCH)]

            for c in range(NCH):
                attn_t[c] = attn_pool.tile([P, T], BF16, tag=f"attn{c}",
                                           name=f"attn{c}")
            d_half = [None, None]
            for i in (0, 1):
                d_half[i] = attn_pool.tile([H // 2, T], F32, tag=f"d{i}",
                                           name=f"d{i}")
                # rows for not-yet-finished pairs must not be NaN: the db
                # selector multiplies them by zero, and 0*NaN = NaN
                nc.vector.memset(d_half[i][:], 1.0)
                rd_half[i] = attn_pool.tile([H // 2, T], F32R, tag=f"rd{i}",
                                            name=f"rd{i}")

            # deferred normalization closures, emitted one (pair, half) late
            # so the db matmul never head-blocks the in-order PE queue
            pending_norm = []

            def emit_pending():
                while pending_norm:
                    pending_norm.pop(0)()

            sq_t = [None] * NCH

            def defer_norm(pair, half):
                g, j = pair // 4, pair % 4
                lo = half * 512

                def go():
                    db = proj_ps.tile([P, 512], F32, tag="pp", name="pp")
                    nc.tensor.matmul(
                        db[:],
                        ind8[:, j * P:(j + 1) * P],
                        rd_half[g][:, lo:lo + 512],
                    )
                    nc.vector.tensor_mul(
                        attn_t[pair][:, lo:lo + 512],
                        attn_t[pair][:, lo:lo + 512],
                        db[:],
                    )
                    if half == 1 and pair < NCH - 2:
                        # squares for the LN stats, while attention still runs
                        sq_t[pair] = sq_pool.tile([P, T], BF16,
                                                  tag=f"sqt{pair}",
                                                  name=f"sqt{pair}")
                        nc.vector.tensor_mul(
                            sq_t[pair][:], attn_t[pair][:], attn_t[pair][:]
                        )
                pending_norm.append(go)

            # projection 3 is queued (not drained): it fills PE gaps in
            # pairs 0-1; each pair then queues pair+4's projection, so
            # filler work is spread across the whole attention phase
            push_qk_proj(3, emit_qk_weights(3))
            for pair in range(NCH):
                drain_until(pair)
                if pair >= 1 and pair + 3 < NCH:
                    push_qk_proj(pair + 3, emit_qk_weights(pair + 3))

                # -- attention for this head pair --
                hA, hB = 2 * pair, 2 * pair + 1
                for half in (0, 1):
                    lo = half * 512
                    aA = at_pool.tile([65, 512], F32, tag="attnA", name="attnA")
                    aB = at_pool.tile([65, 512], F32, tag="attnB", name="attnB")
                    for c in range(NTC):
                        sc = sc_pool.tile([P, 1024], F32, tag="sc", name="sc")
                        nc.tensor.matmul(
                            sc[:, 0:512],
                            kT_t[pair][0:HD, ts(c, P)],
                            qT_t[pair][0:HD, lo:lo + 512],
                        )
                        nc.tensor.matmul(
                            sc[:, 512:1024],
                            kT_t[pair][HD:P, ts(c, P)],
                            qT_t[pair][HD:P, lo:lo + 512],
                        )
                        pr = pr_pool.tile([P, 1024], BF16, tag="pr", name="pr")
                        nc.scalar.activation(pr[:], sc[:], AF.Exp,
                                             scale=exp_scale)
                        em_rep = (em_t[c][:, lo:lo + 512]
                                  .unsqueeze(1).broadcast_to([P, 2, 512]))
                        nc.vector.tensor_mul(
                            pr[:].rearrange("p (c n) -> p c n", c=2),
                            pr[:].rearrange("p (c n) -> p c n", c=2),
                            em_rep,
                        )
                        nc.tensor.matmul(
                            aA[:],
                            v_t[c][:, 66 * hA:66 * hA + 65],
                            pr[:, 0:512],
                            start=(c == 0),
                            stop=(c == NTC - 1),
                        )
                        nc.tensor.matmul(
                            aB[:],
                            v_t[c][:, 66 * hB:66 * hB + 65],
                            pr[:, 512:1024],
                            start=(c == 0),
                            stop=(c == NTC - 1),
                        )
                        # pace the queued projection work across the whole
                        # attention phase (~6 closures per pair)
                        if c % 2 == 0 or len(filler) > 24:
                            drain_filler(1)
                    # emit previous chunk's normalization now: its inputs are
                    # long ready, so it slots into the PE queue without
                    # blocking, ahead of this chunk's dependent ops
                    emit_pending()
                    # denominator rows: extract from PSUM row 64 on the
                    # Scalar engine (engine writes must start 32-aligned, so
                    # arbitrary rd rows are reached via DMA), then recip the
                    # whole 8-row block from the raw values (idempotent) and
                    # round to f32r for the selector matmul
                    g, j = pair // 4, pair % 4
                    dsA = rr_pool.tile([65, 512], F32, tag="dsA", name="dsA")
                    nc.scalar.copy(dsA[64:65, :], aA[64:65, :])
                    nc.gpsimd.dma_start(
                        d_half[g][2 * j:2 * j + 1, lo:lo + 512],
                        dsA[64:65, :],
                    )
                    dsB = rr_pool.tile([65, 512], F32, tag="dsB", name="dsB")
                    nc.scalar.copy(dsB[64:65, :], aB[64:65, :])
                    nc.gpsimd.dma_start(
                        d_half[g][2 * j + 1:2 * j + 2, lo:lo + 512],
                        dsB[64:65, :],
                    )
                    rdt = rr_pool.tile([H // 2, 512], F32, tag="rdt",
                                       name="rdt")
                    nc.vector.reciprocal_approx_fast(
                        out=rdt[:], in_=d_half[g][:, lo:lo + 512],
                    )
                    nc.vector.tensor_copy(rd_half[g][:, lo:lo + 512], rdt[:])
                    # evacuate unnormalized attn on the Scalar engine: it
                    # has slack, and the DVE queue (em-mul, recips) would
                    # delay the PSUM release that gates the next half's PV
                    nc.vector.tensor_copy(attn_t[pair][0:HD, lo:lo + 512],
                                          aA[0:HD, :])
                    nc.vector.tensor_copy(attn_t[pair][HD:P, lo:lo + 512],
                                          aB[0:HD, :])
                    defer_norm(pair, half)
            emit_pending()
            drain_filler()

        # ---------------- LN statistics -------------------------
        def emit_wg(eo):
            wtile = wg_pool.tile([P, 2 * NCH * P], BF16, tag="wg", name="wg")
            nc.sync.dma_start(wtile[:], wg[eo])
            c1t = wg_pool.tile([1, 2 * P], F32R, tag="c1", name="c1")
            nc.sync.dma_start(c1t[:], c1p[eo][None, :])
            return {"w": wtile, "c1": c1t}

        wg_next = emit_wg(0)
        stats_pool = ctx.enter_context(tc.tile_pool(name="stats", bufs=1))
        mu_neg = stats_pool.tile([1, T], F32, tag="mu_neg", name="mu_neg")
        msq = stats_pool.tile([1, T], F32, tag="msq", name="msq")
        var = stats_pool.tile([1, T], F32, tag="var", name="var")
        rstd = stats_pool.tile([1, T], F32, tag="rstd", name="rstd")
        rstdr = stats_pool.tile([1, T], F32R, tag="rstdr", name="rstdr")
        mu_negr = stats_pool.tile([1, T], F32R, tag="mu_negr", name="mu_negr")
        rstd_bc = stats_pool.tile([P, T], F32, tag="rstd_bc", name="rstd_bc")

        with tc.tile_pool(name="db_ps", bufs=4, space="PSUM") as db_pool, \
             tc.tile_pool(name="st_ps", bufs=1, space="PSUM") as st_pool:
            # mu accumulates at partition 0, sq at partition 32 of the same
            # bank (distinct col groups) — two banks total for the stats
            stt = [st_pool.tile([33, 512], F32, tag=f"st{h}", name=f"st{h}")
                   for h in (0, 1)]
            mu_ps = [stt[h][0:1, :] for h in (0, 1)]
            sq_ps = [stt[h][32:33, :] for h in (0, 1)]
            for c in range(NCH - 2, NCH):
                sq_t[c] = sq_pool.tile([P, T], BF16, tag=f"sqt{c}",
                                       name=f"sqt{c}")
                nc.vector.tensor_mul(sq_t[c][:], attn_t[c][:], attn_t[c][:])
            for c in range(NCH):
                for half in (0, 1):
                    lo = half * 512
                    nc.tensor.matmul(
                        mu_ps[half], ones_col[:], attn_t[c][:, lo:lo + 512],
                        start=(c == 0), stop=(c == NCH - 1),
                    )
                    nc.tensor.matmul(
                        sq_ps[half], ones_col[:], sq_t[c][:, lo:lo + 512],
                        start=(c == 0), stop=(c == NCH - 1),
                    )
            for half in (0, 1):
                lo = half * 512
                nc.scalar.mul(mu_neg[0:1, lo:lo + 512], mu_ps[half], -1.0 / E)
                nc.scalar.mul(msq[0:1, lo:lo + 512], sq_ps[half], 1.0 / E)
            nc.vector.tensor_mul(var[:], mu_neg[:], mu_neg[:])
            nc.vector.tensor_tensor(
                var[:], msq[:], var[:], mybir.AluOpType.subtract
            )
            nc.scalar.activation(rstd[:], var[:], AF.Sqrt, bias=epst[:])
            nc.vector.reciprocal_approx_fast(out=rstd[:], in_=rstd[:])
            nc.vector.tensor_copy(rstdr[:], rstd[:])
            nc.vector.tensor_copy(mu_negr[:], mu_neg[:])

            # ---------------- output projection -----------------------------
            # the rstd broadcast matmuls are emitted after eo 0's groups so
            # the o-proj matmuls (which don't need rstd) hide the serial LN
            # scalar chain instead of head-blocking behind the broadcast
            def emit_rstd_bc():
                for half in (0, 1):
                    lo = half * 512
                    rb = db_pool.tile([P, 512], F32, tag="db", name="db")
                    nc.tensor.matmul(
                        rb[:],
                        ones_row[:],
                        rstdr[0:1, lo:lo + 512],
                    )
                    nc.vector.tensor_copy(rstd_bc[:, lo:lo + 512], rb[:])

            deferred_muls = []
            for eo in range(NCH):
                wt = wg_next
                if eo + 1 < NCH:
                    wg_next = emit_wg(eo + 1)

                for half in (0, 1):
                    lo = half * 512
                    ps = db_pool.tile([P, 512], F32, tag="db", name="db")
                    for s0, s1, m in _segs(lo, lo + 512, split):
                        for c in range(NCH):
                            nc.tensor.matmul(
                                ps[:, s0 - lo:s1 - lo],
                                wt["w"][:, m * NCH * P + c * P:
                                        m * NCH * P + (c + 1) * P],
                                attn_t[c][:, s0:s1],
                                start=(c == 0),
                                stop=False,
                            )
                        nc.tensor.matmul(
                            ps[:, s0 - lo:s1 - lo],
                            wt["c1"][0:1, m * P:(m + 1) * P],
                            mu_negr[0:1, s0:s1],
                            start=False,
                            stop=True,
                        )
                    osb = osb_pool.tile([P, 512], F32, tag="osb",
                                        name="osb")

                    def _mul(ps=ps, lo=lo, osb=osb, eo=eo):
                        nc.vector.tensor_mul(
                            osb[:], ps[:], rstd_bc[:, lo:lo + 512]
                        )
                        if o_bias:
                            for s0, s1, m in _segs(lo, lo + 512, split):
                                nc.scalar.activation(
                                    osb[:, s0 - lo:s1 - lo],
                                    osb[:, s0 - lo:s1 - lo], AF.Identity,
                                    bias=c2_sb[:, m * NCH + eo:
                                               m * NCH + eo + 1],
                                )
                        nc.sync.dma_start(outT[ts(eo, P), lo:lo + 512],
                                          osb[:])
                    if eo == 0:
                        deferred_muls.append(_mul)
                        if half == 1:
                            emit_rstd_bc()
                            while deferred_muls:
                                deferred_muls.pop(0)()
                    else:
                        _mul()


    nc.compile()
    return nc


def _pack_pmajor(arr2d, np_dt):
    # [NCH*P, T] -> [P, NCH*T]: row p holds chunk-major concatenation
    return np.ascontiguousarray(
        arr2d.reshape(NCH, P, T).transpose(1, 0, 2).reshape(P, NCH * T)
    ).astype(np_dt)


def _fp8_pair(arr):
    a8 = arr.astype(NPF8)
    ar = (arr - a8.astype(np.float32)).astype(NPF8)
    return a8, ar


def _host_prep(inputs):
    scaling = HD ** -0.5
    f32 = np.float32

    def a(name):
        return np.asarray(inputs[name], f32)

    def prep_blocks_fp8(Wt, Wi):
        # [2, eo, p, c*128+j] with arr[c*128+p, eo*128+j], fp8 main+residual
        o8 = np.empty((2, NCH, P, NCH * P), NPF8)
        orr = np.empty((2, NCH, P, NCH * P), NPF8)
        for m, W in enumerate((Wt, Wi)):
            arr = (W * WS).T.astype(f32)  # [e_in, e_out], scaled
            a8, ar = _fp8_pair(arr)
            for dst, src in ((o8, a8), (orr, ar)):
                dst[m] = (
                    src.reshape(NCH, P, NCH, P)
                    .transpose(2, 1, 0, 3)
                    .reshape(NCH, P, NCH * P)
                )
        return np.ascontiguousarray(o8), np.ascontiguousarray(orr)

    def prep_blocks_bf16(Wt, Wi):
        out = np.empty((2, NCH, P, NCH * P), NPBF16)
        for m, W in enumerate((Wt, Wi)):
            arr = (W.T).astype(NPBF16)
            out[m] = (
                arr.reshape(NCH, P, NCH, P)
                .transpose(2, 1, 0, 3)
                .reshape(NCH, P, NCH * P)
            )
        return np.ascontiguousarray(out)

    Wo_t, Wo_i = a("Wo_t"), a("Wo_i")
    g_t, g_i = a("ln_g_t"), a("ln_g_i")
    b_t, b_i = a("ln_b_t"), a("ln_b_i")
    Wg_t = Wo_t * g_t[None, :]
    Wg_i = Wo_i * g_i[None, :]

    def pack4(o8, orr):
        # [2(m), NCH, P, NCH*P] x2 -> [NCH, P, 4*NCH*P]:
        # per eo [m0-main | m1-main | m0-res | m1-res]
        return np.ascontiguousarray(
            np.concatenate([o8[0], o8[1], orr[0], orr[1]], axis=-1)
        )

    wq_np = pack4(*prep_blocks_fp8(a("Wq_t"), a("Wq_i")))
    wk_np = pack4(*prep_blocks_fp8(a("Wk_t"), a("Wk_i")))
    wg2 = prep_blocks_bf16(Wg_t, Wg_i)
    wg_np = np.ascontiguousarray(np.concatenate([wg2[0], wg2[1]], axis=-1))

    wv8_np = np.empty((2, 2, P, NCH * 512), NPF8)
    wvr_np = np.empty((2, 2, P, NCH * 512), NPF8)
    for m, W in enumerate((a("Wv_t"), a("Wv_i"))):
        arr = (W * WS).T.astype(f32)  # [e_in, e_out], scaled
        a8, ar = _fp8_pair(arr)
        for dst, src in ((wv8_np, a8), (wvr_np, ar)):
            dst[m] = (
                src.reshape(NCH, P, 2, 512)
                .transpose(2, 1, 0, 3)
                .reshape(2, P, NCH * 512)
            )
    # [eoh, P, 2*NCH*512]: per eoh [m0 | m1]
    wv_main_np = np.ascontiguousarray(
        np.concatenate([wv8_np[0], wv8_np[1]], axis=-1)
    )
    wv_res_np = np.ascontiguousarray(
        np.concatenate([wvr_np[0], wvr_np[1]], axis=-1)
    )

    em_np = _pack_pmajor(
        np.exp(np.asarray(inputs["attention_mask"], np.float64)).T.astype(NPBF16),
        NPBF16,
    )

    bq_np = np.stack([a("bq_t"), a("bq_i")]) * f32(WS)
    bk_np = np.stack([a("bk_t"), a("bk_i")]) * f32(WS)
    bv_np = np.stack([a("bv_t"), a("bv_i")]) * f32(WS)
    c1_np = np.stack(
        [Wg_t.astype(np.float64).sum(1), Wg_i.astype(np.float64).sum(1)]
    ).astype(f32)
    # [NCH, 2*P]: per eo [m0-slice | m1-slice]
    c1p_np = np.ascontiguousarray(
        np.concatenate(
            [c1_np[0].reshape(NCH, P), c1_np[1].reshape(NCH, P)], axis=-1
        )
    )
    c2_np = np.stack(
        [
            Wo_t.astype(np.float64) @ b_t.astype(np.float64) + a("bo_t"),
            Wo_i.astype(np.float64) @ b_i.astype(np.float64) + a("bo_i"),
        ]
    ).astype(f32)

    # indp row 2: all-ones (rstd bcast); ind8: 1/(64 d) selector —
    # ind8[k, j*P+m] selects rd rows (2j, 2j+1) -> bcast rows (<64, >=64)
    ones8_np = np.ones((H // 2, T), np.float32)
    indp_np = np.zeros((3, P), np.float32)
    indp_np[2, :] = 1.0
    ind8_np = np.zeros((8, 4 * P), np.float32)
    for j in range(4):
        ind8_np[2 * j, j * P:j * P + HD] = 1.0 / WS
        ind8_np[2 * j + 1, j * P + HD:(j + 1) * P] = 1.0 / WS

    shared = dict(
        wq_all=wq_np, wk_all=wk_np, wg=wg_np,
        wv_main=wv_main_np, wv_res=wv_res_np,
        em=em_np, indp_d=indp_np, ind8_d=ind8_np, ones8_d=ones8_np,
        bq=np.ascontiguousarray(bq_np), bk=np.ascontiguousarray(bk_np),
        bv=np.ascontiguousarray(bv_np), c1=np.ascontiguousarray(c1_np),
        c1p=c1p_np, c2=np.ascontiguousarray(c2_np),
    )
    flags = (
        bool(np.any(bv_np)),
        bool(np.any(bq_np) or np.any(bk_np)),
        bool(np.any(c2_np)),
    )
    return shared, flags


_CACHE = {}


def build_cached(split, flags):
    key = (split, flags)
    if key not in _CACHE:
        _CACHE[key] = build_module(split, *flags)
    return _CACHE[key]


def kernel(**inputs):
    q = np.asarray(inputs["query"], np.float32)
    k = np.asarray(inputs["key"], np.float32)
    v = np.asarray(inputs["value"], np.float32)
    assert q.shape == (B, T, E), q.shape
    split = int(np.asarray(inputs["split_position"]))

    shared, flags = _host_prep(inputs)
    nc = build_cached(split, flags)

    in_maps = []
    for b in range(B):
        m = dict(shared)
        for nm, arr in (("xq", q), ("xk", k), ("xv", v)):
            xt = arr[b].T.astype(np.float32)
            x8 = xt.astype(NPF8)
            xr = (xt - x8.astype(np.float32)).astype(NPF8)
            m[f"{nm}8"] = _pack_pmajor(x8.astype(np.float32), NPF8)
            m[f"{nm}r"] = _pack_pmajor(xr.astype(np.float32), NPF8)
        in_maps.append(m)

    res = run_bass_kernel_spmd(nc, in_maps, list(range(B)))
    out = np.stack(
        [np.ascontiguousarray(res.results[b]["outT"].T) for b in range(B)]
    )
    return out.astype(np.float32)
